# revision 5
# baseline (speedup 1.0000x reference)
"""Trainium2 Bass kernel for nn_FFMLP (4-layer MLP, hidden=128, relu).

V2 strategy (pure data parallel, batch sharded 8 ways):
- Feature-major on-chip layout: activations live as [feat, batch]; every layer
  is one K<=128 fp16 matmul per 512-col chunk (weights stationary, activation
  stream moving; fp32 PSUM).
- Quad-strip input layout [128, B/4] (4 row-tiled K=32 strips) so input DMA
  uses all 128 partitions.
- Layer-phased groups of 8 chunks: the PE runs one layer at a time within a
  group, so stationary weights reload only at phase switches (~12 LdWeights
  per group after band-aware dedup, vs ~4 per chunk fine-grained).
- PSUM is one 8-bank rotation of four [128,1024] (2-bank) block tiles. Each
  block = 2 chunks of one layer; its relu+downcast evacuation is ONE FD=1024
  instruction assigned greedily to ScalarE or VectorE (both read PSUM at
  1 elem/cycle/lane on TRN2; ScalarE 1.2GHz vs VectorE 0.96GHz, so the greedy
  split lands ~53/47) -- the two evac engines are the roofline here.
- L4 (M=16) packs 8 chunks into one block via column tiling (tile_position
  (0,32j)); its matmul pairs are interleaved between the NEXT group's L0
  blocks and its evacuation is split into two FD=512 copies emitted
  mid-phase, so evac demand stays uniform (a solid 8-MM L4 burst starves the
  evac engines ~1.7us). Output is DMA'd quad-packed fp16; the host unpacks.
- Output yt is fp16 (host casts to fp32): halves output DMA.
"""
import sys

if "/opt/trn_rl_repo" not in sys.path:
    sys.path.insert(0, "/opt/trn_rl_repo")

import numpy as np

import concourse.bass as bass
import concourse.mybir as mybir
import concourse.tile as tile

INPUT_DIM = 32
OUTPUT_DIM = 16
HIDDEN = 128
PADDED_OUT = 16
NUM_LAYERS = 4
B = 524288
N_CORES = 8
B_CORE = B // N_CORES  # 65536
CHUNK = 512
N_CHUNKS = B_CORE // CHUNK  # 128
GROUP = 8  # chunks per layer-phase group
N_WARM = 20  # PE p-state warm-up matmuls (FD=128)

fp16 = mybir.dt.float16
fp32 = mybir.dt.float32
RELU = mybir.ActivationFunctionType.Relu

# cost-model ns for the greedy evac balancer (TRN2: ACT 1.2GHz +222cyc init,
# DVE 0.96GHz +120cyc init; both 1 elem/cycle/lane from fp32 PSUM)
ACT_EVAC_NS = (1024 + 222) / 1.2
DVE_EVAC_NS = (1024 + 120) / 0.96


def _split_waits(nc, max_waits=1):
    """walrus in this image rejects >1 semaphore wait per instruction on some
    formats; split excess waits onto preceding NOPs on the same engine queue
    (queues are in-order, so semantics are preserved)."""
    n_new = 0
    for bb in nc.main_func.blocks:
        out_list = []
        changed = False
        for ins in bb.instructions:
            si = ins.sync_info
            if si is not None and si.on_wait and len(si.on_wait) > max_waits:
                waits = list(si.on_wait)
                extra, keep = waits[:-max_waits], waits[-max_waits:]
                while extra:
                    chunk, extra = extra[:max_waits], extra[max_waits:]
                    n_new += 1
                    nop = mybir.InstNoOp(name=f"I-waitsplit-{n_new}", ins=[], outs=[])
                    nop.engine = ins.engine
                    nop.sync_info = mybir.SyncInfo(on_wait=chunk, on_update=[])
                    out_list.append(nop)
                ins.sync_info = mybir.SyncInfo(on_wait=keep, on_update=si.on_update)
                changed = True
            out_list.append(ins)
        if changed:
            bb.instructions = out_list
    return n_new


def _rect_of(ins):
    """PE-array rectangle (r0, r1, c0, c1) occupied by an InstLdweights."""
    tp = ins.tile_position or (0, 0)
    ts = getattr(ins, "tile_size", None) or (128, 128)
    r0, c0 = int(tp[0]), int(tp[1])
    kr, mc = int(ts[0]), int(ts[1])
    return (r0, r0 + kr, c0, c0 + mc)


def _dedup_ldweights(nc):
    """Band-aware LdWeights dedup: the PE array retains weights per tile
    rectangle; a load whose (weights AP, position, size, mode) matches what is
    already resident in that rectangle -- and has not been overlapped by a
    later load -- is replaced with a NOP carrying the same sync_info."""
    n = 0
    for bb in nc.main_func.blocks:
        il = list(bb.instructions)
        resident = {}  # (r0, c0) -> (key, rect)
        changed = False
        for idx, ins in enumerate(il):
            if ins.engine != mybir.EngineType.PE:
                continue
            if isinstance(ins, mybir.InstLdweights):
                rect = _rect_of(ins)
                key = (
                    repr(ins.ins[0]),
                    str(ins.tile_position),
                    str(getattr(ins, "tile_size", None)),
                    str(ins.perf_mode),
                    bool(ins.is_transpose),
                )
                pos = (rect[0], rect[2])
                cur = resident.get(pos)
                if cur is not None and cur[0] == key:
                    nop = mybir.InstNoOp(name=ins.name, ins=[], outs=[])
                    nop.engine = ins.engine
                    nop.sync_info = ins.sync_info
                    il[idx] = nop
                    changed = True
                    n += 1
                    continue
                # evict any resident rectangle this load overlaps
                for p, (k, rc) in list(resident.items()):
                    if rect[0] < rc[1] and rc[0] < rect[1] and rect[2] < rc[3] and rc[2] < rect[3]:
                        del resident[p]
                resident[pos] = (key, rect)
        if changed:
            bb.instructions = il
    return n


def build(n_chunks=N_CHUNKS):
    nc = bass.Bass()
    ncols = n_chunks * CHUNK
    nquad = ncols // 4
    n_groups = n_chunks // GROUP
    assert n_chunks % GROUP == 0 and GROUP == 8

    # xt quad-strip: xt[32*j + f, q*512 + c] = x.T[f, (4q+j)*512 + c]
    xt = nc.declare_dram_parameter("xt", [128, nquad], fp16, isOutput=False)
    w0 = nc.declare_dram_parameter("w0", [128, HIDDEN], fp16, isOutput=False)
    w1 = nc.declare_dram_parameter("w1", [HIDDEN, HIDDEN], fp16, isOutput=False)
    w2 = nc.declare_dram_parameter("w2", [HIDDEN, HIDDEN], fp16, isOutput=False)
    w3 = nc.declare_dram_parameter("w3", [HIDDEN, HIDDEN], fp16, isOutput=False)
    w4 = nc.declare_dram_parameter("w4", [HIDDEN, PADDED_OUT], fp16, isOutput=False)
    # yt quad-packed: yt[32*j + r, q*512 + c] = out[(4q+j)*512 + c, r], r<16
    yt = nc.declare_dram_parameter("yt", [128, nquad], fp16, isOutput=True)

    with tile.TileContext(nc) as tc:
        with (
            tc.tile_pool(name="wp", bufs=1) as wp,
            tc.tile_pool(name="io", bufs=1) as io,
            tc.tile_pool(name="hp", bufs=1) as hp,
            tc.tile_pool(name="op", bufs=1) as op,
            tc.tile_pool(name="ps", bufs=1, space="PSUM") as ps,
        ):
            # HAM warm-up source: memset (no DMA dependency, PE can start
            # ramping immediately)
            wwarm = wp.tile([128, 128], fp16, tag="wm", name="wwarm")
            nc.vector.memset(wwarm[:, :], 0.0)

            w0s = wp.tile([128, HIDDEN], fp16, tag="w0", name="w0s")
            w1s = wp.tile([HIDDEN, HIDDEN], fp16, tag="w1", name="w1s")
            w2s = wp.tile([HIDDEN, HIDDEN], fp16, tag="w2", name="w2s")
            w3s = wp.tile([HIDDEN, HIDDEN], fp16, tag="w3", name="w3s")
            w4s = wp.tile([HIDDEN, PADDED_OUT], fp16, tag="w4", name="w4s")

            def blk_tile(name):
                return ps.tile([128, 1024], fp32, tag="blk", bufs=4, name=name)

            def l4_tile():
                return blk_tile("pl4")

            pwarm = blk_tile("pwarm")
            for _ in range(N_WARM):
                nc.tensor.matmul(
                    pwarm[:, 0:128], wwarm[:, :], wwarm[:, 0:128],
                    start=True, stop=True,
                )

            # greedy two-engine evac balancer
            bal = {"act": 0.0, "dve": 0.0}

            def evac(dst, src, relu, fd=1024):
                act_ns = (fd + 222) / 1.2
                dve_ns = (fd + 120) / 0.96
                use_act = bal["act"] + act_ns <= bal["dve"] + dve_ns
                if use_act:
                    bal["act"] += act_ns
                    if relu:
                        nc.scalar.activation(dst, src, RELU)
                    else:
                        nc.scalar.copy(out=dst, in_=src)
                else:
                    bal["dve"] += dve_ns
                    if relu:
                        nc.vector.tensor_scalar_max(dst, src, 0.0)
                    else:
                        nc.vector.tensor_copy(dst, src)

            slabs = {}

            def load_slab(g, split=False):
                if g >= n_groups:
                    return
                W = GROUP * 128
                xs = io.tile([128, W], fp16, tag="xin", bufs=4, name="xs")
                if split:
                    nc.sync.dma_start(out=xs[:, : W // 2], in_=xt[:, g * W : g * W + W // 2])
                    nc.sync.dma_start(out=xs[:, W // 2 :], in_=xt[:, g * W + W // 2 : (g + 1) * W])
                else:
                    nc.sync.dma_start(out=xs, in_=xt[:, g * W : (g + 1) * W])
                slabs[g] = xs

            def l4_mm(blk, pairs, i):
                """One L4 matmul: chunk i of its group into [128,1024] block
                rows 32j (j=i%4), col half qh=i//4."""
                j, qh = i % 4, i // 4
                src = pairs[i // 2]
                nc.tensor.matmul(
                    blk[32 * j : 32 * j + PADDED_OUT, qh * 512 : (qh + 1) * 512],
                    w4s[:, :],
                    src[:, (i % 2) * 512 : (i % 2 + 1) * 512],
                    start=True,
                    stop=True,
                    tile_position=(0, 32 * j),
                )

            def l4_finish(g, blk):
                osb = op.tile([128, 1024], fp16, tag="osb", bufs=4, name="osb")
                evac(osb[:, :], blk[:, :], relu=False)
                q0 = 2 * g
                nc.sync.dma_start(
                    out=yt[:, q0 * 512 : (q0 + 2) * 512], in_=osb[:, :]
                )

            # DMA order: first input slab first (the long pole for the first
            # real matmul), weights interleaved in first-use order.
            load_slab(0)
            nc.sync.dma_start(out=w0s, in_=w0[:, :])
            nc.sync.dma_start(out=w1s, in_=w1[:, :])
            load_slab(1)
            nc.sync.dma_start(out=w2s, in_=w2[:, :])
            nc.sync.dma_start(out=w3s, in_=w3[:, :])
            nc.sync.dma_start(out=w4s, in_=w4[:, :])
            h4_prev = None

            for g in range(n_groups):
                load_slab(g + 2)
                xs = slabs.pop(g)
                h = {}  # (layer, pair) -> SBUF pair tile [128, 1024]

                # The previous group's 8 L4 matmuls are spread as one pair at
                # the END of each of the four phases, so every phase produces
                # 4 evacs per ~2.1us of PE work -- uniform feed for the two
                # evac engines (a solid 8-MM L4 block starves them ~1.7us).
                l4_blk = l4_tile() if h4_prev is not None else None

                # ---- P0: L0 (K=32 strips)
                for m in range(GROUP // 2):
                    blk = blk_tile("p0")
                    for half in range(2):
                        i = 2 * m + half
                        j, ql = i % 4, i // 4
                        nc.tensor.matmul(
                            blk[:, half * 512 : (half + 1) * 512],
                            w0s[32 * j : 32 * j + INPUT_DIM, :],
                            xs[32 * j : 32 * j + INPUT_DIM, ql * 512 : (ql + 1) * 512],
                            start=True,
                            stop=True,
                            tile_position=(32 * j, 0),
                        )
                    h1 = hp.tile([128, 1024], fp16, tag="h1", bufs=8, name="h1")
                    evac(h1[:, :], blk[:, :], relu=True)
                    h[(1, m)] = h1
                    if l4_blk is not None:
                        l4_mm(l4_blk, h4_prev, 2 * m)
                        l4_mm(l4_blk, h4_prev, 2 * m + 1)
                        if m == 1:
                            l4_osb = op.tile([128, 1024], fp16, tag="osb", bufs=4, name="osb")
                            evac(l4_osb[:, 0:512], l4_blk[:, 0:512], relu=False, fd=512)
                if l4_blk is not None:
                    evac(l4_osb[:, 512:1024], l4_blk[:, 512:1024], relu=False, fd=512)
                    q0 = 2 * (g - 1)
                    nc.sync.dma_start(out=yt[:, q0 * 512 : (q0 + 2) * 512], in_=l4_osb[:, :])
                    l4_blk = None

                # ---- P1..P3: L1..L3 (K=128)
                for layer, ws in ((1, w1s), (2, w2s), (3, w3s)):
                    for m in range(GROUP // 2):
                        blk = blk_tile(f"p{layer}")
                        src = h[(layer, m)]
                        for half in range(2):
                            nc.tensor.matmul(
                                blk[:, half * 512 : (half + 1) * 512],
                                ws[:, :],
                                src[:, half * 512 : (half + 1) * 512],
                                start=True,
                                stop=True,
                            )
                        hn = hp.tile(
                            [128, 1024], fp16, tag=f"h{layer + 1}", bufs=8,
                            name=f"h{layer + 1}",
                        )
                        evac(hn[:, :], blk[:, :], relu=True)
                        h[(layer + 1, m)] = hn

                h4_prev = [h[(4, m)] for m in range(GROUP // 2)]

            l4_blk = l4_tile()
            q0 = 2 * (n_groups - 1)
            for i in range(GROUP):
                l4_mm(l4_blk, h4_prev, i)
            osbA = op.tile([128, 512], fp16, tag="osA", bufs=1, name="osbA")
            osbB = op.tile([128, 512], fp16, tag="osB", bufs=1, name="osbB")
            nc.vector.tensor_copy(osbA[:, :], l4_blk[:, 0:512])
            nc.scalar.copy(out=osbB[:, :], in_=l4_blk[:, 512:1024])
            nc.sync.dma_start(out=yt[:, q0 * 512 : (q0 + 1) * 512], in_=osbA[:, :])
            nc.sync.dma_start(out=yt[:, (q0 + 1) * 512 : (q0 + 2) * 512], in_=osbB[:, :])
    _dedup_ldweights(nc)
    _split_waits(nc)
    return nc


def _split_weights(weights):
    ws = []
    off = 0
    ws.append(weights[off : off + HIDDEN * INPUT_DIM].reshape(HIDDEN, INPUT_DIM))
    off += HIDDEN * INPUT_DIM
    for _ in range(NUM_LAYERS - 1):
        ws.append(weights[off : off + HIDDEN * HIDDEN].reshape(HIDDEN, HIDDEN))
        off += HIDDEN * HIDDEN
    ws.append(weights[off : off + PADDED_OUT * HIDDEN].reshape(PADDED_OUT, HIDDEN))
    return ws


_NC_CACHE = {}


def make_in_maps(inputs: np.ndarray, weights: np.ndarray):
    ws = _split_weights(np.asarray(weights, dtype=np.float32))
    w0t = np.ascontiguousarray(ws[0].T).astype(np.float16)  # [32, 128]
    wmaps = {
        "w0": np.concatenate([w0t] * 4, axis=0),  # [128, 128], 4 strips
        "w1": np.ascontiguousarray(ws[1].T).astype(np.float16),
        "w2": np.ascontiguousarray(ws[2].T).astype(np.float16),
        "w3": np.ascontiguousarray(ws[3].T).astype(np.float16),
        "w4": np.ascontiguousarray(ws[4].T).astype(np.float16),
    }
    in_maps = []
    for i in range(N_CORES):
        xc = inputs[i * B_CORE : (i + 1) * B_CORE]
        xtc = np.ascontiguousarray(xc.T).astype(np.float16)  # [32, B_CORE]
        # quad-strip: [128, B_CORE/4]
        xq = np.ascontiguousarray(
            xtc.reshape(INPUT_DIM, N_CHUNKS // 4, 4, CHUNK)
            .transpose(2, 0, 1, 3)
            .reshape(128, B_CORE // 4)
        )
        in_maps.append({"xt": xq, **wmaps})
    return in_maps


def kernel(inputs: np.ndarray, weights: np.ndarray) -> np.ndarray:
    from concourse.bass_utils import run_bass_kernel_spmd

    assert inputs.shape == (B, INPUT_DIM), inputs.shape
    in_maps = make_in_maps(inputs, weights)
    if "nc" not in _NC_CACHE:
        _NC_CACHE["nc"] = build()
    nc = _NC_CACHE["nc"]
    res = run_bass_kernel_spmd(nc, in_maps, list(range(N_CORES)))
    outs = []
    for r in res.results:
        yq = r["yt"]  # [128, B_CORE/4] fp16 quad-packed
        o = (
            yq.reshape(4, 32, N_CHUNKS // 4, CHUNK)[:, :PADDED_OUT]
            .transpose(2, 0, 3, 1)
            .reshape(B_CORE, PADDED_OUT)
        )
        outs.append(o.astype(np.float32))
    return np.concatenate(outs, axis=0)[:, :OUTPUT_DIM]


# revision 8
# speedup vs baseline: 1.0059x; 1.0059x over previous
"""Trainium2 Bass kernel for nn_FFMLP (4-layer MLP, hidden=128, relu).

V2 strategy (pure data parallel, batch sharded 8 ways):
- Feature-major on-chip layout: activations live as [feat, batch]; every layer
  is one K<=128 fp16 matmul per 512-col chunk (weights stationary, activation
  stream moving; fp32 PSUM).
- Quad-strip input layout [128, B/4] (4 row-tiled K=32 strips) so input DMA
  uses all 128 partitions.
- Layer-phased groups of 8 chunks: the PE runs one layer at a time within a
  group, so stationary weights reload only at phase switches (~12 LdWeights
  per group after band-aware dedup, vs ~4 per chunk fine-grained).
- PSUM is one 8-bank rotation of four [128,1024] (2-bank) block tiles. Each
  block = 2 chunks of one layer; its relu+downcast evacuation is ONE FD=1024
  instruction assigned greedily to ScalarE or VectorE (both read PSUM at
  1 elem/cycle/lane on TRN2; ScalarE 1.2GHz vs VectorE 0.96GHz, so the greedy
  split lands ~53/47) -- the two evac engines are the roofline here.
- L4 (M=16) packs 8 chunks into one block via column tiling (tile_position
  (0,32j)); its matmul pairs are interleaved between the NEXT group's L0
  blocks and its evacuation is split into two FD=512 copies emitted
  mid-phase, so evac demand stays uniform (a solid 8-MM L4 burst starves the
  evac engines ~1.7us). Output is DMA'd quad-packed fp16; the host unpacks.
- Output yt is fp16 (host casts to fp32): halves output DMA.
"""
import sys

if "/opt/trn_rl_repo" not in sys.path:
    sys.path.insert(0, "/opt/trn_rl_repo")

import numpy as np

import concourse.bass as bass
import concourse.mybir as mybir
import concourse.tile as tile

INPUT_DIM = 32
OUTPUT_DIM = 16
HIDDEN = 128
PADDED_OUT = 16
NUM_LAYERS = 4
B = 524288
N_CORES = 8
B_CORE = B // N_CORES  # 65536
CHUNK = 512
N_CHUNKS = B_CORE // CHUNK  # 128
GROUP = 8  # chunks per layer-phase group
N_WARM = 20  # PE p-state warm-up matmuls (FD=128)

fp16 = mybir.dt.float16
fp32 = mybir.dt.float32
RELU = mybir.ActivationFunctionType.Relu

# cost-model ns for the greedy evac balancer (TRN2: ACT 1.2GHz +222cyc init,
# DVE 0.96GHz +120cyc init; both 1 elem/cycle/lane from fp32 PSUM)
ACT_EVAC_NS = (1024 + 222) / 1.2
DVE_EVAC_NS = (1024 + 120) / 0.96


def _split_waits(nc, max_waits=1):
    """walrus in this image rejects >1 semaphore wait per instruction on some
    formats; split excess waits onto preceding NOPs on the same engine queue
    (queues are in-order, so semantics are preserved)."""
    n_new = 0
    for bb in nc.main_func.blocks:
        out_list = []
        changed = False
        for ins in bb.instructions:
            si = ins.sync_info
            if si is not None and si.on_wait and len(si.on_wait) > max_waits:
                waits = list(si.on_wait)
                extra, keep = waits[:-max_waits], waits[-max_waits:]
                while extra:
                    chunk, extra = extra[:max_waits], extra[max_waits:]
                    n_new += 1
                    nop = mybir.InstNoOp(name=f"I-waitsplit-{n_new}", ins=[], outs=[])
                    nop.engine = ins.engine
                    nop.sync_info = mybir.SyncInfo(on_wait=chunk, on_update=[])
                    out_list.append(nop)
                ins.sync_info = mybir.SyncInfo(on_wait=keep, on_update=si.on_update)
                changed = True
            out_list.append(ins)
        if changed:
            bb.instructions = out_list
    return n_new


def _rect_of(ins):
    """PE-array rectangle (r0, r1, c0, c1) occupied by an InstLdweights."""
    tp = ins.tile_position or (0, 0)
    ts = getattr(ins, "tile_size", None) or (128, 128)
    r0, c0 = int(tp[0]), int(tp[1])
    kr, mc = int(ts[0]), int(ts[1])
    return (r0, r0 + kr, c0, c0 + mc)


def _dedup_ldweights(nc):
    """Band-aware LdWeights dedup: the PE array retains weights per tile
    rectangle; a load whose (weights AP, position, size, mode) matches what is
    already resident in that rectangle -- and has not been overlapped by a
    later load -- is replaced with a NOP carrying the same sync_info."""
    n = 0
    for bb in nc.main_func.blocks:
        il = list(bb.instructions)
        resident = {}  # (r0, c0) -> (key, rect)
        changed = False
        for idx, ins in enumerate(il):
            if ins.engine != mybir.EngineType.PE:
                continue
            if isinstance(ins, mybir.InstLdweights):
                rect = _rect_of(ins)
                key = (
                    repr(ins.ins[0]),
                    str(ins.tile_position),
                    str(getattr(ins, "tile_size", None)),
                    str(ins.perf_mode),
                    bool(ins.is_transpose),
                )
                pos = (rect[0], rect[2])
                cur = resident.get(pos)
                if cur is not None and cur[0] == key:
                    nop = mybir.InstNoOp(name=ins.name, ins=[], outs=[])
                    nop.engine = ins.engine
                    nop.sync_info = ins.sync_info
                    il[idx] = nop
                    changed = True
                    n += 1
                    continue
                # evict any resident rectangle this load overlaps
                for p, (k, rc) in list(resident.items()):
                    if rect[0] < rc[1] and rc[0] < rect[1] and rect[2] < rc[3] and rc[2] < rect[3]:
                        del resident[p]
                resident[pos] = (key, rect)
        if changed:
            bb.instructions = il
    return n


def build(n_chunks=N_CHUNKS):
    nc = bass.Bass()
    ncols = n_chunks * CHUNK
    nquad = ncols // 4
    n_groups = n_chunks // GROUP
    assert n_chunks % GROUP == 0 and GROUP == 8

    # xt quad-strip: xt[32*j + f, q*512 + c] = x.T[f, (4q+j)*512 + c]
    xt = nc.declare_dram_parameter("xt", [128, nquad], fp16, isOutput=False)
    w0 = nc.declare_dram_parameter("w0", [128, HIDDEN], fp16, isOutput=False)
    w1 = nc.declare_dram_parameter("w1", [HIDDEN, HIDDEN], fp16, isOutput=False)
    w2 = nc.declare_dram_parameter("w2", [HIDDEN, HIDDEN], fp16, isOutput=False)
    w3 = nc.declare_dram_parameter("w3", [HIDDEN, HIDDEN], fp16, isOutput=False)
    w4 = nc.declare_dram_parameter("w4", [HIDDEN, PADDED_OUT], fp16, isOutput=False)
    # yt quad-packed: yt[32*j + r, q*512 + c] = out[(4q+j)*512 + c, r], r<16
    yt = nc.declare_dram_parameter("yt", [128, nquad], fp16, isOutput=True)

    with tile.TileContext(nc) as tc:
        with (
            tc.tile_pool(name="wp", bufs=1) as wp,
            tc.tile_pool(name="io", bufs=1) as io,
            tc.tile_pool(name="hp", bufs=1) as hp,
            tc.tile_pool(name="op", bufs=1) as op,
            tc.tile_pool(name="ps", bufs=1, space="PSUM") as ps,
        ):
            # HAM warm-up source: memset (no DMA dependency, PE can start
            # ramping immediately)
            wwarm = wp.tile([128, 128], fp16, tag="wm", name="wwarm")
            nc.vector.memset(wwarm[:, :], 0.0)

            w0s = wp.tile([128, HIDDEN], fp16, tag="w0", name="w0s")
            w1s = wp.tile([HIDDEN, HIDDEN], fp16, tag="w1", name="w1s")
            w2s = wp.tile([HIDDEN, HIDDEN], fp16, tag="w2", name="w2s")
            w3s = wp.tile([HIDDEN, HIDDEN], fp16, tag="w3", name="w3s")
            w4s = wp.tile([HIDDEN, PADDED_OUT], fp16, tag="w4", name="w4s")

            def blk_tile(name):
                return ps.tile([128, 1024], fp32, tag="blk", bufs=4, name=name)

            def l4_tile():
                return blk_tile("pl4")

            pwarm = blk_tile("pwarm")
            for _ in range(N_WARM):
                nc.tensor.matmul(
                    pwarm[:, 0:128], wwarm[:, :], wwarm[:, 0:128],
                    start=True, stop=True,
                )

            # greedy two-engine evac balancer
            bal = {"act": 0.0, "dve": 0.0}

            def evac(dst, src, relu, fd=1024):
                act_ns = (fd + 222) / 1.2
                dve_ns = (fd + 120) / 0.96
                use_act = bal["act"] + act_ns <= bal["dve"] + dve_ns
                if use_act:
                    bal["act"] += act_ns
                    if relu:
                        nc.scalar.activation(dst, src, RELU)
                    else:
                        nc.scalar.copy(out=dst, in_=src)
                else:
                    bal["dve"] += dve_ns
                    if relu:
                        nc.vector.tensor_scalar_max(dst, src, 0.0)
                    else:
                        nc.vector.tensor_copy(dst, src)

            slabs = {}

            def load_slab(g):
                if g >= n_groups:
                    return
                W = GROUP * 128
                xs = io.tile([128, W], fp16, tag="xin", bufs=6, name="xs")
                nc.sync.dma_start(out=xs, in_=xt[:, g * W : (g + 1) * W])
                slabs[g] = xs

            def l4_mm(blk, pairs, i):
                """One L4 matmul: chunk i of its group into [128,1024] block
                rows 32j (j=i%4), col half qh=i//4."""
                j, qh = i % 4, i // 4
                src = pairs[i // 2]
                nc.tensor.matmul(
                    blk[32 * j : 32 * j + PADDED_OUT, qh * 512 : (qh + 1) * 512],
                    w4s[:, :],
                    src[:, (i % 2) * 512 : (i % 2 + 1) * 512],
                    start=True,
                    stop=True,
                    tile_position=(0, 32 * j),
                )

            # DMA order: first input slab first (the long pole for the first
            # real matmul), weights interleaved in first-use order.
            load_slab(0)
            nc.sync.dma_start(out=w0s, in_=w0[:, :])
            nc.sync.dma_start(out=w1s, in_=w1[:, :])
            load_slab(1)
            nc.sync.dma_start(out=w2s, in_=w2[:, :])
            nc.sync.dma_start(out=w3s, in_=w3[:, :])
            nc.sync.dma_start(out=w4s, in_=w4[:, :])
            h4_prev = None
            h1_carry = None

            for g in range(n_groups):
                load_slab(g + 2)
                xs = slabs.pop(g)
                h = {}  # (layer, pair) -> SBUF pair tile [128, 1024]

                # The previous group's 8 L4 matmuls are interleaved between
                # this group's L0 blocks with split mid-phase copies, keeping
                # evac production uniform (a solid 8-MM L4 block starves the
                # two evac engines ~1.7us).
                l4_blk = l4_tile() if h4_prev is not None else None

                # ---- P0: L0 (K=32 strips); block 0 may have been carried
                # into the previous group's P3 (levels PE load: P0 was
                # PE-bound, P3 evac-bound)
                def l0_block(m, xs_, into):
                    blk = blk_tile("p0")
                    for half in range(2):
                        i = 2 * m + half
                        j, ql = i % 4, i // 4
                        nc.tensor.matmul(
                            blk[:, half * 512 : (half + 1) * 512],
                            w0s[32 * j : 32 * j + INPUT_DIM, :],
                            xs_[32 * j : 32 * j + INPUT_DIM, ql * 512 : (ql + 1) * 512],
                            start=True,
                            stop=True,
                            tile_position=(32 * j, 0),
                        )
                    h1 = hp.tile([128, 1024], fp16, tag="h1", bufs=16, name="h1")
                    evac(h1[:, :], blk[:, :], relu=True)
                    into[(1, m)] = h1

                m0 = 0
                if h1_carry is not None:
                    h[(1, 0)] = h1_carry
                    h1_carry = None
                    m0 = 1
                n_l4 = 0
                l4_osb = None
                for m in range(m0, GROUP // 2):
                    l0_block(m, xs, h)
                    if l4_blk is not None:
                        take = min(2 if m >= GROUP // 2 - 2 else 1, 4 - n_l4)
                        for _ in range(take):
                            l4_mm(l4_blk, h4_prev, 2 * n_l4)
                            l4_mm(l4_blk, h4_prev, 2 * n_l4 + 1)
                            n_l4 += 1
                            if n_l4 == 2:
                                l4_osb = op.tile(
                                    [128, 1024], fp16, tag="osb", bufs=8, name="osb"
                                )
                                evac(l4_osb[:, 0:512], l4_blk[:, 0:512], relu=False, fd=512)
                if l4_blk is not None:
                    while n_l4 < 4:
                        l4_mm(l4_blk, h4_prev, 2 * n_l4)
                        l4_mm(l4_blk, h4_prev, 2 * n_l4 + 1)
                        n_l4 += 1
                    evac(l4_osb[:, 512:1024], l4_blk[:, 512:1024], relu=False, fd=512)
                    q0 = 2 * (g - 1)
                    nc.sync.dma_start(out=yt[:, q0 * 512 : (q0 + 2) * 512], in_=l4_osb[:, :])
                    l4_blk = None

                # ---- P1..P3: L1..L3 (K=128)
                for layer, ws in ((1, w1s), (2, w2s), (3, w3s)):
                    for m in range(GROUP // 2):
                        blk = blk_tile(f"p{layer}")
                        src = h[(layer, m)]
                        for half in range(2):
                            nc.tensor.matmul(
                                blk[:, half * 512 : (half + 1) * 512],
                                ws[:, :],
                                src[:, half * 512 : (half + 1) * 512],
                                start=True,
                                stop=True,
                            )
                        hn = hp.tile(
                            [128, 1024], fp16, tag=f"h{layer + 1}", bufs=16,
                            name=f"h{layer + 1}",
                        )
                        evac(hn[:, :], blk[:, :], relu=True)
                        h[(layer + 1, m)] = hn

                if g + 1 < n_groups:
                    carry = {}
                    l0_block(0, slabs[g + 1], carry)
                    h1_carry = carry[(1, 0)]
                h4_prev = [h[(4, m)] for m in range(GROUP // 2)]

            l4_blk = l4_tile()
            q0 = 2 * (n_groups - 1)
            for i in range(GROUP):
                l4_mm(l4_blk, h4_prev, i)
            osbA = op.tile([128, 512], fp16, tag="osA", bufs=1, name="osbA")
            osbB = op.tile([128, 512], fp16, tag="osB", bufs=1, name="osbB")
            nc.vector.tensor_copy(osbA[:, :], l4_blk[:, 0:512])
            nc.scalar.copy(out=osbB[:, :], in_=l4_blk[:, 512:1024])
            nc.sync.dma_start(out=yt[:, q0 * 512 : (q0 + 1) * 512], in_=osbA[:, :])
            nc.sync.dma_start(out=yt[:, (q0 + 1) * 512 : (q0 + 2) * 512], in_=osbB[:, :])
    _dedup_ldweights(nc)
    _split_waits(nc)
    return nc


def _split_weights(weights):
    ws = []
    off = 0
    ws.append(weights[off : off + HIDDEN * INPUT_DIM].reshape(HIDDEN, INPUT_DIM))
    off += HIDDEN * INPUT_DIM
    for _ in range(NUM_LAYERS - 1):
        ws.append(weights[off : off + HIDDEN * HIDDEN].reshape(HIDDEN, HIDDEN))
        off += HIDDEN * HIDDEN
    ws.append(weights[off : off + PADDED_OUT * HIDDEN].reshape(PADDED_OUT, HIDDEN))
    return ws


_NC_CACHE = {}


def make_in_maps(inputs: np.ndarray, weights: np.ndarray):
    ws = _split_weights(np.asarray(weights, dtype=np.float32))
    w0t = np.ascontiguousarray(ws[0].T).astype(np.float16)  # [32, 128]
    wmaps = {
        "w0": np.concatenate([w0t] * 4, axis=0),  # [128, 128], 4 strips
        "w1": np.ascontiguousarray(ws[1].T).astype(np.float16),
        "w2": np.ascontiguousarray(ws[2].T).astype(np.float16),
        "w3": np.ascontiguousarray(ws[3].T).astype(np.float16),
        "w4": np.ascontiguousarray(ws[4].T).astype(np.float16),
    }
    in_maps = []
    for i in range(N_CORES):
        xc = inputs[i * B_CORE : (i + 1) * B_CORE]
        xtc = np.ascontiguousarray(xc.T).astype(np.float16)  # [32, B_CORE]
        # quad-strip: [128, B_CORE/4]
        xq = np.ascontiguousarray(
            xtc.reshape(INPUT_DIM, N_CHUNKS // 4, 4, CHUNK)
            .transpose(2, 0, 1, 3)
            .reshape(128, B_CORE // 4)
        )
        in_maps.append({"xt": xq, **wmaps})
    return in_maps


def kernel(inputs: np.ndarray, weights: np.ndarray) -> np.ndarray:
    from concourse.bass_utils import run_bass_kernel_spmd

    assert inputs.shape == (B, INPUT_DIM), inputs.shape
    in_maps = make_in_maps(inputs, weights)
    if "nc" not in _NC_CACHE:
        _NC_CACHE["nc"] = build()
    nc = _NC_CACHE["nc"]
    res = run_bass_kernel_spmd(nc, in_maps, list(range(N_CORES)))
    outs = []
    for r in res.results:
        yq = r["yt"]  # [128, B_CORE/4] fp16 quad-packed
        o = (
            yq.reshape(4, 32, N_CHUNKS // 4, CHUNK)[:, :PADDED_OUT]
            .transpose(2, 0, 3, 1)
            .reshape(B_CORE, PADDED_OUT)
        )
        outs.append(o.astype(np.float32))
    return np.concatenate(outs, axis=0)[:, :OUTPUT_DIM]
